# revision 29
# baseline (speedup 1.0000x reference)
"""Trainium2 Bass kernel for the DiagonalSSMBlock problem.

Math (per batch, sharded one batch per core over 8 cores):
    a = -exp(log_neg_real) + i*imag ; a_bar = exp(a) = r * e^{i theta}
    b_bar = ((a_bar-1)/a)[:,None] * B
    Bu_t = b_bar @ u_t                         (complex, state dim 64)
    h_t = a_bar * h_{t-1} + Bu_t               (diagonal complex scan over L)
    y_t = C @ Re(h_t) + D*u_t ; out = LN(u + y) * gamma + beta   (C is real)

The problem is HBM-bound; the error gate (2e-2) leaves ~1e4x precision
headroom over an fp32 implementation, so all large tensors move as bf16:
  * u ships twice in bf16 (natural layout for the residual/LN, transposed
    for the Bu matmul contraction over d_model) = 16 MiB instead of 32.
  * out ships as bf16 and is upcast on host (8 MiB instead of 16).
  * cos/sin rotation tables ship as bf16 (2 MiB instead of 4).
Per-core traffic ~27 MiB vs ~53 MiB for the fp32 hi/lo variant.

Device decomposition per l-tile of 512 timesteps:
  * Bu lands in scan layout [re|im states on 128 partitions, L free] via a
    packed [b_re; b_im]^T stationary operand (single bf16 pass, 8 K-chunks).
  * The complex scan is rotated into a per-lane REAL damped scan:
    g_t = r*g_{t-1} + w_t with w_t = e^{-i theta t} Bu_t (elementwise
    rotation against host cos/sin tables), h_re_t = Re(e^{i theta t} g_t).
    The real scan maps to one DVE tensor_tensor_scan per 512-wide slice,
    chained via its initial value.  Rotation elementwise ops are bf16 so
    DVE runs them in 2x mode; multiplies are split DVE/Pool.
  * Readout y = h_re^T @ C^T as plain bf16 matmuls (K=64) per 128-step
    subtile.  Residual + LayerNorm: DVE scalar_tensor_tensor computes
    x=y+u and accumulates sum(x); ACT Square accumulates sum(x^2); the
    normalize pass is split between ACT (activation scale/bias) and Pool
    (tensor_scalar) to balance engine load.
  * The loop is software-pipelined three tiles deep (Bu of tile i is
    emitted before the scan/readout of tile i-3).
  * DMA is batched: one transfer per l-tile each for u / uth / out.
"""

import numpy as np

import concourse.mybir as mybir
import concourse.tile as tile
from concourse import bacc, bass_utils
from concourse.bass import MemorySpace
from concourse.mybir import ActivationFunctionType as act
from concourse.mybir import AluOpType as alu

F32 = mybir.dt.float32
BF16 = mybir.dt.bfloat16
P = 128          # partitions
L = 4096         # sequence length per core
DM = 1024        # d_model
NS = 64          # d_state
LT = 512         # l-tile (scan slice, matmul moving width)
NSUB = LT // P   # 4 l-subtiles of 128 rows per l-tile
NT = L // LT     # 8 l-tiles
KC = DM // P     # 8 contraction chunks of 128
NCORES = 8
LN_EPS = 1e-5
DH = 512         # d-model half (psum bank width)


def _build_program(use_ures: bool, use_gb: bool):
    """Builds the single-core Bass/Tile program (SPMD across 8 cores)."""
    nc = bacc.Bacc("TRN2", num_devices=NCORES, debug=False)

    u_d = nc.dram_tensor("u", [L, DM], BF16, kind="ExternalInput").ap()
    uth_d = nc.dram_tensor("uth", [DM, L], BF16, kind="ExternalInput").ap()
    bbh_d = nc.dram_tensor("bbh", [P, DM], BF16, kind="ExternalInput").ap()
    cth_d = nc.dram_tensor("cth", [NS, DM], BF16, kind="ExternalInput").ap()
    trig_d = nc.dram_tensor("trig", [P, L], BF16, kind="ExternalInput").ap()
    trigb_d = nc.dram_tensor("trigb", [P, L], BF16, kind="ExternalInput").ap()
    rt_d = nc.dram_tensor("rt", [P, LT], BF16, kind="ExternalInput").ap()
    ures_d = (
        nc.dram_tensor("ures", [L, DM], F32, kind="ExternalInput").ap()
        if use_ures
        else None
    )
    if use_gb:
        gam_d = nc.dram_tensor("gam", [P, DM], F32, kind="ExternalInput").ap()
        bet_d = nc.dram_tensor("bet", [P, DM], F32, kind="ExternalInput").ap()
    out_d = nc.dram_tensor("out", [L, DM], BF16, kind="ExternalOutput").ap()

    # batched-DMA views: [p, s, d] with l = s*128 + p
    u_v = u_d.rearrange("(s p) d -> p s d", p=P)
    ur_v = ures_d.rearrange("(s p) d -> p s d", p=P) if use_ures else None
    out_v = out_d.rearrange("(s p) d -> p s d", p=P)
    # [p, c, l] with d = c*128 + p
    uth_v = uth_d.rearrange("(c p) l -> p c l", p=P)

    with tile.TileContext(nc) as tc:
        with (
            tc.tile_pool(name="singles", bufs=1) as singles,
            tc.tile_pool(name="u", bufs=3) as u_pool,
            tc.tile_pool(name="ur", bufs=3) as ur_pool,
            tc.tile_pool(name="ut", bufs=3) as ut_pool,
            tc.tile_pool(name="tg", bufs=3) as tg_pool,
            tc.tile_pool(name="w", bufs=2) as w_pool,
            tc.tile_pool(name="g", bufs=3) as g_pool,
            tc.tile_pool(name="h", bufs=3) as h_pool,
            tc.tile_pool(name="x", bufs=5) as x_pool,
            tc.tile_pool(name="tmp", bufs=2) as tmp_pool,
            tc.tile_pool(name="sq", bufs=2) as sq_pool,
            tc.tile_pool(name="o", bufs=2) as o_pool,
            tc.tile_pool(name="st", bufs=3) as st_pool,
            tc.tile_pool(name="pb", bufs=2, space=MemorySpace.PSUM) as psum_b,
            tc.tile_pool(name="py", bufs=3, space=MemorySpace.PSUM) as psum_y,
        ):
            # Singles are allocated up front but their DMAs are interleaved
            # with the first uth tiles so the first Bu matmul starts ASAP.
            bbh_s = singles.tile([P, DM], BF16)
            cth_s = singles.tile([NS, DM], BF16)
            rt_s = singles.tile([P, LT], BF16)
            eps_s = singles.tile([P, 1], F32)
            nc.gpsimd.memset(eps_s[:], LN_EPS)
            if use_gb:
                gam_s = singles.tile([P, DM], F32)
                bet_s = singles.tile([P, DM], F32)

            g_prev = None
            dma_stash = {}
            trg_stash = {}
            bu_stash = {}
            b1_stash = {}
            for it in range(NT + 3):
                # ---- stage A0: prefetch uT tile `it` ----
                if it < NT:
                    l0 = it * LT
                    th_t = ut_pool.tile([P, KC, LT], BF16, tag="uth")
                    nc.sync.dma_start(th_t[:], uth_v[:, :, l0 : l0 + LT])
                    tg_t = tg_pool.tile([P, LT], BF16, tag="tg")
                    nc.sync.dma_start(tg_t[:], trig_d[:, l0 : l0 + LT])
                    tgb_t = tg_pool.tile([P, LT], BF16, tag="tgb")
                    nc.sync.dma_start(tgb_t[:], trigb_d[:, l0 : l0 + LT])
                    dma_stash[it] = th_t
                    trg_stash[it] = (tg_t, tgb_t)
                    if it == 0:
                        nc.sync.dma_start(bbh_s[:], bbh_d)
                        nc.sync.dma_start(cth_s[:], cth_d)
                        nc.sync.dma_start(rt_s[:], rt_d)
                        if use_gb:
                            nc.sync.dma_start(gam_s[:], gam_d)
                            nc.sync.dma_start(bet_s[:], bet_d)

                # ---- stage A1: Bu matmul for tile `it-1` ----
                at = it - 1
                if 0 <= at < NT:
                    th_t = dma_stash.pop(at)
                    bu = psum_b.tile([P, LT], F32, tag="bu")
                    for k in range(KC):
                        nc.tensor.matmul(
                            bu[:],
                            bbh_s[:, k * P : (k + 1) * P],
                            th_t[:, k, :],
                            start=(k == 0),
                            stop=(k == KC - 1),
                        )
                    bu_stash[at] = bu

                # ---- stage B1: rotation + scan for tile `it-2` ----
                jt = it - 2
                if not (0 <= jt < NT):
                    jt = None
                if jt is not None:
                    bu = bu_stash.pop(jt)
                    l0 = jt * LT
                    u_t = u_pool.tile([P, NSUB, DM], BF16, tag="u")
                    nc.sync.dma_start(
                        u_t[:], u_v[:, NSUB * jt : NSUB * (jt + 1), :]
                    )
                    if use_ures:
                        ur_t = ur_pool.tile([P, NSUB, DM], F32, tag="ur")
                        nc.sync.dma_start(
                            ur_t[:], ur_v[:, NSUB * jt : NSUB * (jt + 1), :]
                        )
                    else:
                        ur_t = u_t

                    # trig: cos on parts 0-63, sin on 64-127.
                    # trigb holds [sin; -cos] so BOTH combine steps below
                    # are ADDs (DVE 2x mode); the scan then carries -g_im
                    # on the im lanes, and the post-rotation is a pure ADD.
                    tg_t, tgb_t = trg_stash.pop(jt)
                    cs_lo = tg_t[0:NS, :]
                    sn_hi = tg_t[NS:P, :]
                    sn_lo = tgb_t[0:NS, :]
                    ncs_hi = tgb_t[NS:P, :]

                    # pre-rotation: w = e^{-i theta t} * Bu.  POOL cannot
                    # touch PSUM, so its mults read a bf16 SBUF copy (ACT);
                    # the DVE mults read PSUM directly.
                    bs = w_pool.tile([P, LT], BF16, tag="bs")
                    nc.scalar.copy(bs[:], bu[:])
                    w = w_pool.tile([P, LT], BF16, tag="w")
                    t1 = tmp_pool.tile([NS, LT], BF16, tag="t1")
                    t2 = tmp_pool.tile([NS, LT], BF16, tag="t2")
                    nc.gpsimd.tensor_tensor(t1[:], bs[0:NS, :], cs_lo, alu.mult)
                    nc.vector.tensor_tensor(t2[:], bu[NS:P, :], sn_hi, alu.mult)
                    nc.vector.tensor_tensor(w[0:NS, :], t1[:], t2[:], alu.add)
                    t3 = tmp_pool.tile([NS, LT], BF16, tag="t1")
                    t4 = tmp_pool.tile([NS, LT], BF16, tag="t2")
                    nc.gpsimd.tensor_tensor(t3[:], bs[NS:P, :], ncs_hi, alu.mult)
                    nc.vector.tensor_tensor(t4[:], bu[0:NS, :], sn_lo, alu.mult)
                    nc.vector.tensor_tensor(w[NS:P, :], t3[:], t4[:], alu.add)

                    # damped real scan (DVE, all-bf16 for 2x mode), chained
                    # across l-tiles; internal scan state stays fp32.
                    g = g_pool.tile([P, LT], BF16, tag="g")
                    init = 0.0 if g_prev is None else g_prev[:, LT - 1 : LT]
                    nc.vector.tensor_tensor_scan(
                        g[:], rt_s[:], w[:], init, alu.mult, alu.add
                    )
                    g_prev = g

                    # post-rotation h_re = cos*g_re + sin*(-g_im), as bf16
                    # for the readout matmul stationary operand.
                    t5 = tmp_pool.tile([NS, LT], BF16, tag="t1")
                    t6 = tmp_pool.tile([NS, LT], BF16, tag="t2")
                    nc.gpsimd.tensor_tensor(t5[:], g[0:NS, :], cs_lo, alu.mult)
                    nc.gpsimd.tensor_tensor(t6[:], g[NS:P, :], sn_hi, alu.mult)
                    hreb = h_pool.tile([NS, LT], BF16, tag="hreb")
                    nc.vector.tensor_tensor(hreb[:], t5[:], t6[:], alu.add)
                    b1_stash[jt] = (hreb, ur_t)

                # ---- stage B2: readout + LN for tile `it-3` ----
                kt = it - 3
                if kt < 0:
                    continue
                hreb, ur_t = b1_stash.pop(kt)

                # readout (K=64 matmuls) into a 2-bank PSUM tile per
                # l-subtile; one DVE stt forms x=y+u with sum(x) accum.
                # Square (sum x^2) on ACT.
                sx = st_pool.tile([P, NSUB], F32, tag="sx")
                sq = st_pool.tile([P, NSUB], F32, tag="sq")
                x_list = []
                for ls in range(NSUB):
                    hsl = hreb[:, ls * P : (ls + 1) * P]
                    x = x_pool.tile([P, DM], F32, tag="x")
                    y_p = psum_y.tile([P, DM], F32, tag="y")
                    for dh in range(2):
                        sl = slice(dh * DH, (dh + 1) * DH)
                        nc.tensor.matmul(y_p[:, sl], hsl, cth_s[:, sl])
                    nc.vector.scalar_tensor_tensor(
                        x[:],
                        y_p[:],
                        1.0,
                        ur_t[:, ls, :],
                        alu.mult,
                        alu.add,
                        accum_out=sx[:, ls : ls + 1],
                    )
                    sqs = sq_pool.tile([P, DM], F32, tag="sqs")
                    nc.scalar.activation(
                        sqs[:], x[:], act.Square, accum_out=sq[:, ls : ls + 1]
                    )
                    x_list.append(x)

                # LN stats for the 4 l-subtiles (fused):
                # vraw = sq - sx^2/DM ; sd = sqrt(vraw/DM + eps)
                # rstd = 1/sd ; nmr = -sx*rstd/DM
                ss = st_pool.tile([P, NSUB], F32, tag="ss")
                nc.gpsimd.tensor_tensor(ss[:], sx[:], sx[:], alu.mult)
                vraw = st_pool.tile([P, NSUB], F32, tag="vraw")
                nc.vector.scalar_tensor_tensor(
                    vraw[:], ss[:], -1.0 / DM, sq[:], alu.mult, alu.add
                )
                sd = st_pool.tile([P, NSUB], F32, tag="sd")
                nc.scalar.activation(
                    sd[:], vraw[:], act.Sqrt, bias=eps_s[:, 0:1], scale=1.0 / DM
                )
                rstd = st_pool.tile([P, NSUB], F32, tag="rstd")
                nc.vector.reciprocal(rstd[:], sd[:])
                nmr = st_pool.tile([P, NSUB], F32, tag="nmr")
                nc.vector.scalar_tensor_tensor(
                    nmr[:], sx[:], -1.0 / DM, rstd[:], alu.mult, alu.mult
                )

                # normalize o = x*rstd + (-mu*rstd), split ACT/Pool; batched
                # bf16 store
                o_t = o_pool.tile([P, NSUB, DM], BF16, tag="o")
                for ls in range(NSUB):
                    if ls < 2:
                        nc.scalar.activation(
                            o_t[:, ls, :],
                            x_list[ls][:],
                            act.Identity,
                            bias=nmr[:, ls : ls + 1],
                            scale=rstd[:, ls : ls + 1],
                        )
                    else:
                        nc.gpsimd.tensor_scalar(
                            o_t[:, ls, :],
                            x_list[ls][:],
                            rstd[:, ls : ls + 1],
                            nmr[:, ls : ls + 1],
                            alu.mult,
                            alu.add,
                        )
                    if use_gb:
                        nc.vector.tensor_tensor(
                            o_t[:, ls, :], o_t[:, ls, :], gam_s[:], alu.mult
                        )
                        nc.vector.tensor_tensor(
                            o_t[:, ls, :], o_t[:, ls, :], bet_s[:], alu.add
                        )
                    if ls == 1:
                        nc.sync.dma_start(
                            out_v[:, NSUB * kt : NSUB * kt + 2, :], o_t[:, 0:2, :]
                        )
                nc.sync.dma_start(
                    out_v[:, NSUB * kt + 2 : NSUB * (kt + 1), :], o_t[:, 2:4, :]
                )
    nc.compile()
    return nc


try:
    import ml_dtypes

    ml_bf16 = ml_dtypes.bfloat16
except ImportError:  # pragma: no cover
    ml_bf16 = None


def _host_params(log_neg_real, imag, B_mat, C_mat):
    lnr = np.asarray(log_neg_real, np.float64)
    im = np.asarray(imag, np.float64)
    a = -np.exp(lnr) + 1j * im
    a_bar = np.exp(a)
    r = np.abs(a_bar)
    b_bar = ((a_bar - 1.0) / a)[:, None] * np.asarray(B_mat, np.float64)
    b_re = np.real(b_bar).astype(np.float32)
    b_im = np.imag(b_bar).astype(np.float32)
    # packed stationary operand for the Bu matmul: [K=d, M=128(re|im)] laid out
    # in SBUF as [128 partitions, KC*128] with chunk k at columns k*128:(k+1)*128
    bbT = np.concatenate([b_re, b_im], axis=0).T  # (DM, 128)
    bb = np.ascontiguousarray(
        bbT.reshape(KC, P, P).transpose(1, 0, 2).reshape(P, DM)
    )
    bbh = bb.astype(ml_bf16)
    cth = np.ascontiguousarray(np.asarray(C_mat, np.float32).T).astype(ml_bf16)
    t = np.arange(L, dtype=np.float64)
    ang = (im[:, None] * t[None, :]) % (2 * np.pi)
    cosT = np.cos(ang).astype(ml_bf16)
    sinT = np.sin(ang).astype(ml_bf16)
    trig = np.ascontiguousarray(np.concatenate([cosT, sinT], axis=0))  # (128, L)
    # [sin; -cos]: makes the w_im combine an ADD producing -w_im, so the
    # scan's im lanes carry -g_im and the post-rotation is cos*g_re +
    # sin*(-g_im) — every DVE combine is an ADD (2x mode eligible).
    trigb = np.ascontiguousarray(np.concatenate([sinT, -cosT], axis=0))
    rfull = np.concatenate([r, r]).astype(ml_bf16)
    rt = np.ascontiguousarray(np.broadcast_to(rfull[:, None], (P, LT)))
    return bbh, cth, trig, trigb, rt


def _make_in_maps(u, log_neg_real, imag, B_mat, C_mat, D, gamma, beta):
    u = np.asarray(u, np.float32)
    Dv = np.asarray(D, np.float32)
    gam = np.asarray(gamma, np.float32)
    bet = np.asarray(beta, np.float32)
    use_ures = bool(np.any(Dv != 0.0))
    use_gb = bool(np.any(gam != 1.0) or np.any(bet != 0.0))

    bbh, cth, trig, trigb, rt = _host_params(log_neg_real, imag, B_mat, C_mat)
    shared = {
        "bbh": bbh,
        "cth": cth,
        "trig": trig,
        "trigb": trigb,
        "rt": rt,
    }
    if use_gb:
        shared["gam"] = np.ascontiguousarray(
            np.broadcast_to(gam[None, :], (P, DM)).astype(np.float32)
        )
        shared["bet"] = np.ascontiguousarray(
            np.broadcast_to(bet[None, :], (P, DM)).astype(np.float32)
        )
    in_maps = []
    for b in range(NCORES):
        m = dict(shared)
        ub = np.ascontiguousarray(u[b])
        m["u"] = np.ascontiguousarray(ub.astype(ml_bf16))
        m["uth"] = np.ascontiguousarray(ub.T.astype(ml_bf16))
        if use_ures:
            m["ures"] = np.ascontiguousarray(ub * (1.0 + Dv)[None, :])
        in_maps.append(m)
    return use_ures, use_gb, in_maps


_PROGRAM_CACHE = {}


def kernel(u, log_neg_real, imag, B_mat, C_mat, D, gamma, beta):
    use_ures, use_gb, in_maps = _make_in_maps(
        u, log_neg_real, imag, B_mat, C_mat, D, gamma, beta
    )
    key = (use_ures, use_gb)
    if key not in _PROGRAM_CACHE:
        _PROGRAM_CACHE[key] = _build_program(use_ures, use_gb)
    nc = _PROGRAM_CACHE[key]

    res = bass_utils.run_bass_kernel_spmd(nc, in_maps, core_ids=list(range(NCORES)))
    return np.stack(
        [np.asarray(r["out"]).astype(np.float32) for r in res.results], axis=0
    )
